# revision 9
# baseline (speedup 1.0000x reference)
"""AlignFeatureLayer Trainium2 kernel, 8-core data-parallel over batch.

reference (per batch b):
    d_proj = relu(d_emb @ W.T + b)          [LD, E]
    q_proj = relu(q_emb @ W.T + b)          [LQ, E]
    scores = d_proj @ q_proj.T              [LD, LQ]
    scores = where(q_mask, -inf, scores)
    alpha  = softmax(scores, axis=-1)       [LD, LQ]
    align  = alpha @ q_emb                  [LD, E]
returns (align, alpha)

Sharding: batch 32 -> 4 per core x 8 cores. No collectives.

Compute layout per core (all on-chip tensors [E|m|l]-major so every matmul
contracts over the partition dim):
  - projections computed transposed: q_projT/d_projT[e, m|l] = relu(W_T.T @ embT)
    via f32r matmuls (full 1-cycle/row rate, ~6e-5 rel err vs fp32)
  - scores[l-tile, m] accumulated over 6 e-chunks; softmax row-wise:
    DVE negated row-max -> ACT exp(bias=-max) -> fused DVE mask-mul+row-sum ->
    reciprocal; alpha = (exp * 1/sum) * mask fused on DVE
  - exp tile transposed on PE (bf16, via identity) -> align = emT.T @ q_emb
    (bf16) accumulated over m-chunks, scaled by 1/sum in the ACT epilogue
Host side: W pre-transposed, embeddings pre-transposed to [E, L] layout,
q_emb also passed natively in bf16 for the align contraction, bool mask
converted to {0,1} float multiplier.
"""
import sys

if '/opt/trn_rl_repo' not in sys.path:
    sys.path.insert(0, '/opt/trn_rl_repo')

import ml_dtypes
import numpy as np

import concourse.bass as bass
import concourse.mybir as mybir
import concourse.tile as tile
from concourse import bacc
from concourse.bass_utils import run_bass_kernel_spmd
from concourse.masks import make_identity

F32 = mybir.dt.float32
F32R = mybir.dt.float32r
BF16 = mybir.dt.bfloat16
AF = mybir.ActivationFunctionType
ALU = mybir.AluOpType

B, LD, LQ, E = 32, 2048, 512, 768
NCORES = 8
NB = B // NCORES          # batches per core
EC = E // 128             # 6 e/d chunks
NLC = LD // 512           # 4 l-chunks per batch
NLT = 4                   # 4 l-tiles of 128 per l-chunk
NMC = LQ // 128           # 4 m-chunks

_CACHED = None


def _build():
    nc = bacc.Bacc(None, target_bir_lowering=False)

    w_t = nc.declare_dram_parameter("w_t", [EC, 128, E], F32, isOutput=False)
    b_col = nc.declare_dram_parameter("b_col", [128, EC], F32, isOutput=False)
    q_embT = nc.declare_dram_parameter("q_embT", [NB, EC, 128, LQ], F32, isOutput=False)
    d_embT = nc.declare_dram_parameter("d_embT", [NB, EC, 128, LD], F32, isOutput=False)
    q_emb = nc.declare_dram_parameter("q_emb", [NB, NMC, 128, E], BF16, isOutput=False)
    maskmul = nc.declare_dram_parameter("maskmul", [NB, 1, LQ], F32, isOutput=False)
    ones_r = nc.declare_dram_parameter("ones_r", [1, 128], F32, isOutput=False)

    align_o = nc.declare_dram_parameter("align", [NB, LD, E], F32, isOutput=True)
    alpha_o = nc.declare_dram_parameter("alpha", [NB, LD, LQ], F32, isOutput=True)

    with tile.TileContext(nc) as tc:
        with (
            tc.tile_pool(name="const", bufs=1) as cpool,
            tc.tile_pool(name="qside", bufs=2) as qpool,
            tc.tile_pool(name="dside", bufs=2) as dpool,
            tc.tile_pool(name="soft", bufs=3) as spool,
            tc.tile_pool(name="stats", bufs=6) as stpool,
            tc.tile_pool(name="pp", bufs=3, space="PSUM") as pp,
            tc.tile_pool(name="pa", bufs=2, space="PSUM") as pa,
            tc.tile_pool(name="ptp", bufs=1, space="PSUM") as ptp,
        ):
            # batch-0 q-side chunks interleaved with W chunks; the first
            # matmul group consumes (w0,qT0) so those stream first
            w_c = []
            qT0 = []
            for dc in range(EC):
                w = cpool.tile([128, E], F32R, tag=f"w{dc}")
                nc.sync.dma_start(w[:], w_t[dc].bitcast(F32R))
                w_c.append(w)
                q = qpool.tile([128, LQ], F32R, tag=f"qT{dc}")
                nc.sync.dma_start(q[:], q_embT[0, dc].bitcast(F32R))
                qT0.append(q)
            ones_sb = cpool.tile([1, 128], F32R)
            nc.sync.dma_start(ones_sb[:], ones_r[:].bitcast(F32R))
            b_sb = cpool.tile([128, EC], F32)
            nc.sync.dma_start(b_sb[:], b_col[:].bitcast(F32))

            ident = cpool.tile([128, 128], F32)
            make_identity(nc, ident[:])
            ident_b = cpool.tile([128, 128], BF16)
            nc.vector.tensor_copy(ident_b[:], ident[:])

            for bi in range(NB):
                # ---- load q-side for this batch ----
                mk = qpool.tile([1, LQ], F32R, tag="mk")
                nc.sync.dma_start(mk[:], maskmul[bi].bitcast(F32R))
                if bi == 0:
                    qT = qT0
                else:
                    qT = []
                    for dc in range(EC):
                        q = qpool.tile([128, LQ], F32R, tag=f"qT{dc}")
                        nc.sync.dma_start(q[:], q_embT[bi, dc].bitcast(F32R))
                        qT.append(q)
                # ---- q_projT[e, m] = relu(W_T.T @ q_embT + b) ----
                qp = qpool.tile([128, EC, LQ], F32R, tag="qp")
                for ec in range(EC):
                    ps = pp.tile([128, LQ], F32, tag="pp")
                    for dc in range(EC):
                        nc.tensor.matmul(
                            ps[:], w_c[dc][:, ec * 128:(ec + 1) * 128],
                            qT[dc][:], start=(dc == 0), stop=(dc == EC - 1))
                    nc.scalar.activation(qp[:, ec, :], ps[:], AF.Relu,
                                         bias=b_sb[:, ec:ec + 1])

                # maskrep[128, LQ] = broadcast of maskmul across partitions
                mrp = pp.tile([128, LQ], F32, tag="pp")
                nc.tensor.matmul(mrp[:], ones_sb[:], mk[:], start=True, stop=True)
                maskrep = qpool.tile([128, LQ], F32, tag="maskrep")
                nc.vector.tensor_copy(maskrep[:], mrp[:])

                for lc in range(NLC):
                    l0 = lc * 512
                    # ---- load d_embT l-chunk ----
                    dch = []
                    for dc in range(EC):
                        d = dpool.tile([128, 512], F32R, tag=f"dch{dc}")
                        nc.sync.dma_start(
                            d[:], d_embT[bi, dc, :, l0:l0 + 512].bitcast(F32R))
                        dch.append(d)
                    if lc == 0:
                        # q_emb needed only from the first align matmul
                        qe = qpool.tile([128, NMC, E], BF16, tag="qe")
                        for mc in range(NMC):
                            nc.sync.dma_start(qe[:, mc, :], q_emb[bi, mc])
                    # ---- d_projT[e, l] for this chunk ----
                    dp = dpool.tile([128, EC, 512], F32R, tag="dp")
                    for ec in range(EC):
                        ps = pp.tile([128, 512], F32, tag="pp")
                        for dc in range(EC):
                            nc.tensor.matmul(
                                ps[:], w_c[dc][:, ec * 128:(ec + 1) * 128],
                                dch[dc][:], start=(dc == 0), stop=(dc == EC - 1))
                        nc.scalar.activation(dp[:, ec, :], ps[:], AF.Relu,
                                             bias=b_sb[:, ec:ec + 1])

                    for lt in range(NLT):
                        t0 = lt * 128
                        # ---- scores[l-tile, m] ----
                        sp = pp.tile([128, LQ], F32, tag="pp")
                        for ec in range(EC):
                            nc.tensor.matmul(
                                sp[:], dp[:, ec, t0:t0 + 128], qp[:, ec, :],
                                start=(ec == 0), stop=(ec == EC - 1))
                        # exp(s - rowmax); rowmax over all cols (incl. masked) is
                        # >= true max, so exp <= 1 and the shift cancels in alpha
                        negmax = stpool.tile([128, 1], F32, tag="negmax")
                        nc.vector.tensor_reduce(
                            negmax[:], sp[:], axis=mybir.AxisListType.X,
                            op=ALU.max, negate=True)
                        em0 = spool.tile([128, LQ], F32, tag="em0")
                        nc.scalar.activation(em0[:], sp[:], AF.Exp, bias=negmax[:])
                        # masked exp (bf16, feeds transpose+align) + row-sum
                        em_b = spool.tile([128, LQ], BF16, tag="em_b")
                        rsum = stpool.tile([128, 1], F32, tag="rsum")
                        nc.vector.scalar_tensor_tensor(
                            em_b[:], em0[:], 1.0, maskrep[:],
                            op0=ALU.mult, op1=ALU.mult, accum_out=rsum[:])
                        recip = stpool.tile([128, 1], F32, tag="recip")
                        nc.vector.reciprocal(recip[:], rsum[:])
                        # alpha out = (em0 * recip) * maskrep, fused on DVE
                        al_t = spool.tile([128, LQ], F32, tag="al_t")
                        nc.vector.scalar_tensor_tensor(
                            al_t[:], em0[:], recip[:], maskrep[:],
                            op0=ALU.mult, op1=ALU.mult)
                        nc.sync.dma_start(
                            alpha_o[bi, l0 + t0:l0 + t0 + 128, :], al_t[:])
                        # transpose em_b -> emT tiles (4x 128x128 into one
                        # single-bank psum tile, then one DVE copy out)
                        ptt = ptp.tile([128, LQ], BF16, tag="ptp")
                        for mc in range(NMC):
                            nc.tensor.transpose(
                                ptt[:, mc * 128:(mc + 1) * 128],
                                em_b[:, mc * 128:(mc + 1) * 128], ident_b[:])
                        at = spool.tile([128, NMC, 128], BF16, tag="at")
                        nc.vector.tensor_copy(at[:], ptt[:])
                        # align[l-tile, :] = emT.T @ q_emb, scaled by recip
                        ap = pa.tile([128, E], F32, tag="pa")
                        for mc in range(NMC):
                            nc.tensor.matmul(ap[:, 0:512], at[:, mc, :],
                                             qe[:, mc, 0:512],
                                             start=(mc == 0), stop=(mc == NMC - 1))
                            nc.tensor.matmul(ap[:, 512:768], at[:, mc, :],
                                             qe[:, mc, 512:768],
                                             start=(mc == 0), stop=(mc == NMC - 1))
                        ao = spool.tile([128, E], F32, tag="ao")
                        nc.scalar.activation(ao[:], ap[:], AF.Copy, scale=recip[:])
                        nc.sync.dma_start(
                            align_o[bi, l0 + t0:l0 + t0 + 128, :], ao[:])

    nc.compile()
    return nc


def kernel(d_embedding, q_embedding, q_mask, W, b):
    global _CACHED
    if _CACHED is None:
        _CACHED = _build()
    nc = _CACHED

    d_embedding = np.ascontiguousarray(d_embedding, dtype=np.float32)
    q_embedding = np.ascontiguousarray(q_embedding, dtype=np.float32)
    W = np.ascontiguousarray(W, dtype=np.float32)
    b = np.ascontiguousarray(b, dtype=np.float32)

    w_t = np.ascontiguousarray(W.T).reshape(EC, 128, E)
    b_col = np.ascontiguousarray(b.reshape(EC, 128).T)
    ones_r = np.ones((1, 128), dtype=np.float32)
    mask_f = (~q_mask.astype(bool)).astype(np.float32)  # 1 real, 0 pad

    in_maps = []
    for c in range(NCORES):
        bs = slice(c * NB, (c + 1) * NB)
        d_sh = d_embedding[bs]            # [NB, LD, E]
        q_sh = q_embedding[bs]            # [NB, LQ, E]
        in_maps.append(dict(
            w_t=w_t,
            b_col=b_col,
            ones_r=ones_r,
            q_embT=np.ascontiguousarray(q_sh.transpose(0, 2, 1)).reshape(NB, EC, 128, LQ),
            d_embT=np.ascontiguousarray(d_sh.transpose(0, 2, 1)).reshape(NB, EC, 128, LD),
            q_emb=np.ascontiguousarray(q_sh).reshape(NB, NMC, 128, E).astype(ml_dtypes.bfloat16),
            maskmul=np.ascontiguousarray(mask_f[bs]).reshape(NB, 1, LQ),
        ))

    res = run_bass_kernel_spmd(nc, in_maps, core_ids=list(range(NCORES)))

    align = np.concatenate([res.results[c]["align"] for c in range(NCORES)], axis=0)
    alpha = np.concatenate([res.results[c]["alpha"] for c in range(NCORES)], axis=0)
    return align.astype(np.float32), alpha.astype(np.float32)
